# revision 21
# baseline (speedup 1.0000x reference)
"""GCN layer (BN -> dense -> sparse softmax -> gather/scatter -> tanh) on 8
Trainium2 NeuronCores.

Strategy (1D edge parallelism, v2 — on-device one-hot, half-windows):
 - Destination nodes sharded 12500/core; each edge lives on the core owning
   its destination row. Host materializes per-edge-slot SOURCE features plus
   a ones column: xs[slot] = [x[col] | 1] (fp16), laid out per 64-node
   half-window in kw 128-edge chunks.
 - The scatter one-hot is built ON DEVICE (v1 DMA'd it: 58 MB/core of
   zeros): M̃[e, i, c] = (loc[e,c] == i) * exp(v[e,c]), two DVE
   tensor_tensor passes in 2x mode against a host-sent replicated iota.
   Folding exp into M̃ means the 58 MB feature stream goes STRAIGHT from
   DRAM into the PE (v1 spent 240 µs of slow-mode DVE scaling it).
 - Per chunk one PE matmul accumulates A[i, 0:128] += M̃_c^T x[col] and
   A[i, 128] += M̃_c^T 1 = softmax denominator, two 64-node half-windows
   packed into one [128, 129] PSUM tile (partition ranges 0:64 / 64:128).
 - Flush per 128-node pair: A -> fp16, PE-transpose num and den column;
   ps2 = AT^T @ W' + den ⊗ b' (rank-1 bias matmul makes den==0 rows give
   tanh(0) = 0 with no masking); th = Tanh(ps2 * rec) fused on the act
   engine with per-partition scale rec = 1/max(den, eps).
 - BatchNorm folds into the projection: per-core partial sums -> AllReduce
   (the only collective) -> W' = rstd*W, b' = -mean*rstd @ W'.
 - Softmax needs no max subtraction: edge_vals are uniform [0,1).

DMA queues: xs stream on sync; small loads on scalar; stores + collective
staging on gpsimd. Numerics: matmul operands fp16 (PSUM fp32), stats and
softmax denominator fp32.
"""
import sys

sys.path.insert(0, "/opt/trn_rl_repo")

import numpy as np
from contextlib import ExitStack

import concourse.bass as bass
import concourse.bacc as bacc
import concourse.mybir as mybir
import concourse.tile as tile
from concourse.bass_utils import run_bass_kernel_spmd

# problem constants
N = 100000
E = 1600000
F = 128
D = 64
BN_EPS = 1e-3
NCORES = 8
NPC = N // NCORES            # 12500 destination nodes per core
WIN = 64                     # destination nodes per half-window
NW = (NPC + WIN - 1) // WIN  # 196 half-windows per core (last has 20 nodes)
GHW = 6                      # half-windows per processing group

f16, f32 = mybir.dt.float16, mybir.dt.float32

_cache: dict[int, object] = {}
_last_kw: int | None = None


def _groups():
    gs, h = [], 0
    while h < NW:
        g = min(GHW, NW - h)
        gs.append((h, g))
        h += g
    return gs


XBAR = False
LAG_DEFAULT = 16
ABUF = 3
DTPBUF = 1
DUALQ = 'pool'

def _build(kw: int, xbar=None, lag=None):
    """Build the SPMD program. kw = max 128-edge chunks per half-window."""
    xbar = XBAR if xbar is None else xbar
    lag = LAG_DEFAULT if lag is None else lag
    nch = NW * kw                    # chunks per core
    gmax = GHW * kw                  # chunks in a full group

    nc = bacc.Bacc(None, target_bir_lowering=False)

    xT = nc.declare_dram_parameter("xT", [F, NPC], f16, isOutput=False)
    w_in = nc.declare_dram_parameter("w_in", [F, D], f32, isOutput=False)
    ident_in = nc.declare_dram_parameter("ident_in", [128, 128], f16, isOutput=False)
    iota_in = nc.declare_dram_parameter("iota_in", [128, WIN * gmax], f16, isOutput=False)
    loc_in = nc.declare_dram_parameter("loc_in", [128, nch], f16, isOutput=False)
    val_in = nc.declare_dram_parameter("val_in", [128, nch], f16, isOutput=False)
    xs_in = nc.declare_dram_parameter("xs_in", [128, nch * (F + 1)], f16, isOutput=False)
    out_p = nc.declare_dram_parameter("out", [NPC, D], f32, isOutput=True)

    with tile.TileContext(nc) as tc:
        with ExitStack() as ctx:
            sb = ctx.enter_context(tc.tile_pool(name="sb", bufs=1))
            pp = ctx.enter_context(tc.tile_pool(name="pp", bufs=1, space="PSUM"))
            dram = ctx.enter_context(tc.tile_pool(name="dram", bufs=1, space="DRAM"))

            # ---------------- phase 0: BN stats -> W', bias ----------------
            # xT arrives in 4 slices so the reduce/square pipeline with the
            # DMA; the AllReduce (the long pole before wp) starts ~15us
            # earlier than with a monolithic load.
            NSL = 4
            SL = NPC // NSL
            xts = sb.tile([F, NPC], f16)
            parts = sb.tile([F, NSL, 2], f32)
            sq_trash = sb.tile([F, NPC], f16)
            for si in range(NSL):
                sl = slice(si * SL, (si + 1) * SL)
                nc.scalar.dma_start(out=xts[:, sl], in_=xT[:, sl])
                nc.vector.tensor_reduce(
                    out=parts[:, si, 0:1], in_=xts[:, sl],
                    axis=mybir.AxisListType.X, op=mybir.AluOpType.add)
                nc.scalar.activation(
                    out=sq_trash[:, sl], in_=xts[:, sl],
                    func=mybir.ActivationFunctionType.Square,
                    accum_out=parts[:, si, 1:2])
            stats = sb.tile([F, 2], f32)
            nc.vector.tensor_reduce(
                out=stats[:].unsqueeze(2), in_=parts[:].transpose([0, 2, 1]),
                axis=mybir.AxisListType.X, op=mybir.AluOpType.add)

            st_b = dram.tile([F, 2], f32)
            red_b = dram.tile([F, 2], f32)
            nc.gpsimd.dma_start(out=st_b[:], in_=stats[:])
            nc.gpsimd.collective_compute(
                "AllReduce", mybir.AluOpType.add,
                replica_groups=[list(range(NCORES))],
                ins=[st_b[:].opt()], outs=[red_b[:].opt()])
            red = sb.tile([F, 2], f32)
            nc.gpsimd.dma_start(out=red[:], in_=red_b[:])

            mean = sb.tile([F, 1], f32)
            nc.vector.tensor_scalar_mul(out=mean[:], in0=red[:, 0:1], scalar1=1.0 / N)
            ex2 = sb.tile([F, 1], f32)
            nc.vector.tensor_scalar_mul(out=ex2[:], in0=red[:, 1:2], scalar1=1.0 / N)
            msq = sb.tile([F, 1], f32)
            nc.vector.tensor_tensor(out=msq[:], in0=mean[:], in1=mean[:],
                                    op=mybir.AluOpType.mult)
            varep = sb.tile([F, 1], f32)
            nc.vector.tensor_tensor(out=varep[:], in0=ex2[:], in1=msq[:],
                                    op=mybir.AluOpType.subtract)
            nc.vector.tensor_scalar_add(out=varep[:], in0=varep[:], scalar1=BN_EPS)
            sdev = sb.tile([F, 1], f32)
            nc.scalar.activation(out=sdev[:], in_=varep[:],
                                 func=mybir.ActivationFunctionType.Sqrt)
            rstd = sb.tile([F, 1], f32)
            nc.vector.reciprocal(out=rstd[:], in_=sdev[:])

            w_sb = sb.tile([F, D], f32)
            nc.scalar.dma_start(out=w_sb[:], in_=w_in[:])
            wp = sb.tile([F, D], f16)
            nc.vector.tensor_scalar(out=wp[:], in0=w_sb[:], scalar1=rstd[:, 0:1],
                                    scalar2=None, op0=mybir.AluOpType.mult)
            nmr = sb.tile([F, 1], f32)
            nc.vector.tensor_tensor(out=nmr[:], in0=mean[:], in1=rstd[:],
                                    op=mybir.AluOpType.mult)
            nmr16 = sb.tile([F, 1], f16)
            nc.vector.tensor_scalar_mul(out=nmr16[:], in0=nmr[:], scalar1=-1.0)

            # b16 = -mean*rstd @ W' is emitted lazily (right before the first
            # stage-2 drain) so its matmul doesn't sit at the head of the
            # in-order PE queue gating all chunk matmuls on the AllReduce.
            b16_box = []

            def emit_b16():
                b_ps = pp.tile([128, D], f32, tag="init", bufs=1)
                nc.tensor.matmul(out=b_ps[:1, :], lhsT=nmr16[:], rhs=wp[:],
                                 start=True, stop=True)
                b16 = sb.tile([1, D], f16)
                nc.vector.tensor_copy(out=b16[:], in_=b_ps[:1, :])
                b16_box.append(b16)

            # ---------------- phase 1: edges ----------------
            val_sb = sb.tile([128, nch], f16)
            nc.scalar.dma_start(out=val_sb[:], in_=val_in[:])
            loc_sb = sb.tile([128, nch], f16)
            nc.scalar.dma_start(out=loc_sb[:], in_=loc_in[:])
            iota_sb = sb.tile([128, WIN, gmax], f16)
            nc.scalar.dma_start(out=iota_sb[:], in_=iota_in[:])
            ident_sb = sb.tile([128, 128], f16)
            nc.scalar.dma_start(out=ident_sb[:], in_=ident_in[:])
            exp_sb = sb.tile([128, nch], f16)
            nc.scalar.activation(out=exp_sb[:], in_=val_sb[:],
                                 func=mybir.ActivationFunctionType.Exp)

            # Stage 2 of the flush (projection ps2 = AT^T @ W' + den*b', tanh,
            # store) depends on the AllReduce-derived wp/b16. Engine queues
            # execute in order, so emitting stage 2 inline would stall every
            # queue behind the collective for ~60 us. Instead stage 1 banks
            # ATs/drow/rec in deep SBUF rings and stage 2 drains with a lag.
            pending = []

            def stage2(ent):
                pr, m, ATs_t, drow_t, rec_t = ent
                ps2 = pp.tile([128, D], f32, tag="ps2", bufs=2)
                nc.tensor.matmul(out=ps2[:], lhsT=ATs_t[:], rhs=wp[:],
                                 start=True, stop=False)
                nc.tensor.matmul(out=ps2[:], lhsT=drow_t[:], rhs=b16_box[0][:],
                                 start=False, stop=True)
                th = sb.tile([128, D], f32, tag="th", bufs=4)
                nc.scalar.activation(
                    out=th[:], in_=ps2[:],
                    func=mybir.ActivationFunctionType.Tanh,
                    scale=rec_t[:, 0:1])
                nc.gpsimd.dma_start(out=out_p[pr * 128:pr * 128 + m, :],
                                    in_=th[:m, :])

            groups = _groups()
            NG = len(groups)
            LAG_G = lag
            for g, (hw0, ghw) in enumerate(groups):
                ch0 = hw0 * kw
                gch = ghw * kw
                # Alternate the 58 MB feature stream across two DMA queues so
                # transfers overlap (one queue tops out ~275 GB/s on HW).
                # Separate tag rings so a queue never blocks on the other's
                # buffer being freed.
                # Dual-queue the feature stream, but keep early groups off the
                # pool queue: the AllReduce occupies it for ~60 us and any xs
                # load queued behind it would starve the PE (FIFO queues).
                use_pool = DUALQ and g % 2 == 1 and g >= 17
                xs = sb.tile([128, gch, F + 1], f16,
                             tag=f"xs{1 if use_pool else 0}", bufs=2)
                qeng = (nc.gpsimd if DUALQ == 'pool' else nc.scalar) if use_pool else nc.sync
                qeng.dma_start(
                    out=xs[:], in_=xs_in[:, ch0 * (F + 1):(ch0 + gch) * (F + 1)])
                mt = sb.tile([128, WIN, gch], f16, tag="mt", bufs=2)
                nc.vector.tensor_tensor(
                    out=mt[:],
                    in0=loc_sb[:, ch0:ch0 + gch].unsqueeze(1).to_broadcast(
                        [128, WIN, gch]),
                    in1=iota_sb[:, :, 0:gch],
                    op=mybir.AluOpType.is_equal)
                nc.vector.tensor_tensor(
                    out=mt[:], in0=mt[:],
                    in1=exp_sb[:, ch0:ch0 + gch].unsqueeze(1).to_broadcast(
                        [128, WIN, gch]),
                    op=mybir.AluOpType.mult)
                for j in range(ghw):
                    h = hw0 + j
                    side = h % 2
                    if side == 0:
                        A = pp.tile([128, F + 1], f32, tag="A", bufs=ABUF)
                    for c in range(kw):
                        lc = j * kw + c
                        nc.tensor.matmul(
                            out=A[side * WIN:(side + 1) * WIN, :],
                            lhsT=mt[:, :, lc], rhs=xs[:, lc, :],
                            start=(c == 0), stop=(c == kw - 1))
                    if side == 1:
                        pr = h // 2
                        m = min(128, NPC - pr * 128)
                        As = sb.tile([128, F + 1], f16, tag="As", bufs=4)
                        nc.scalar.activation(
                            out=As[:], in_=A[:],
                            func=mybir.ActivationFunctionType.Copy)
                        ATs = sb.tile([128, F], f16, tag="ATs", bufs=98)
                        if xbar:
                            nc.scalar.dma_start_transpose(out=ATs[:], in_=As[:, 0:F])
                        else:
                            ATp = pp.tile([128, F], f16, tag="ATp", bufs=1)
                            nc.tensor.transpose(out=ATp[:], in_=As[:, 0:F],
                                                identity=ident_sb[:])
                            nc.scalar.activation(
                                out=ATs[:], in_=ATp[:],
                                func=mybir.ActivationFunctionType.Copy)
                        dtp = pp.tile([1, 128], f16, tag="dtp", bufs=DTPBUF)
                        nc.tensor.transpose(out=dtp[:], in_=As[:, F:F + 1],
                                            identity=ident_sb[:])
                        drow = sb.tile([1, 128], f16, tag="drow", bufs=98)
                        nc.scalar.activation(
                            out=drow[:], in_=dtp[:],
                            func=mybir.ActivationFunctionType.Copy)
                        dmax = sb.tile([128, 1], f32, tag="dmax", bufs=4)
                        nc.vector.tensor_scalar_max(out=dmax[:], in0=A[:, F:F + 1],
                                                    scalar1=1e-30)
                        rec = sb.tile([128, 1], f32, tag="rec", bufs=98)
                        nc.vector.reciprocal(out=rec[:], in_=dmax[:])
                        pending.append((pr, m, ATs, drow, rec))
                if g >= LAG_G and pending:
                    if not b16_box:
                        emit_b16()
                    k = -(-len(pending) // max(1, NG - 2 - g))
                    for ent in pending[:k]:
                        stage2(ent)
                    del pending[:k]
            if pending and not b16_box:
                emit_b16()
            for ent in pending:
                stage2(ent)

    nc.finalize()
    return nc


def _prep(x, w, edge_vals, rows, cols, kw):
    """Host-side shard/layout construction. Returns in_maps or None if kw
    is too small for this edge distribution."""
    nch = NW * kw
    gmax = GHW * kw

    order = np.argsort(rows, kind="stable")
    rs = rows[order].astype(np.int64)
    cs = cols[order].astype(np.int64)
    vs = edge_vals[order]

    core = rs // NPC
    lic = rs % NPC
    hwin = lic // WIN
    loc = lic % WIN

    run = core * NW + hwin               # global half-window id, monotone in rs
    nruns = NCORES * NW
    counts = np.bincount(run, minlength=nruns)
    if counts.max() > kw * 128:
        return None
    starts = np.zeros(nruns, np.int64)
    np.cumsum(counts[:-1], out=starts[1:])
    pos = np.arange(len(run)) - starts[run]

    chunk = hwin * kw + pos // 128       # chunk index within the core
    e_part = pos % 128

    locf = np.full((NCORES, 128, nch), -1.0, np.float16)
    valf = np.full((NCORES, 128, nch), -100.0, np.float16)
    colf = np.zeros((NCORES, 128, nch), np.int64)
    locf[core, e_part, chunk] = loc.astype(np.float16)
    valf[core, e_part, chunk] = vs
    colf[core, e_part, chunk] = cs

    x16 = x.astype(np.float16)
    ident = np.eye(128, dtype=np.float16)
    iota = np.ascontiguousarray(np.broadcast_to(
        np.arange(WIN, dtype=np.float16)[None, :, None],
        (128, WIN, gmax))).reshape(128, WIN * gmax)
    w32 = np.ascontiguousarray(w.astype(np.float32))
    in_maps = []
    for c in range(NCORES):
        xs = np.empty((128, nch, F + 1), np.float16)
        xs[:, :, 0:F] = x16[colf[c]]
        xs[:, :, F] = 1.0
        xsh = np.ascontiguousarray(x16[c * NPC:(c + 1) * NPC, :].T)
        in_maps.append({
            "xT": xsh,
            "w_in": w32,
            "ident_in": ident,
            "iota_in": iota,
            "loc_in": np.ascontiguousarray(locf[c]),
            "val_in": np.ascontiguousarray(valf[c]),
            "xs_in": xs.reshape(128, nch * (F + 1)),
        })
    return in_maps


def kernel(x, kernel, edge_vals, rows, cols, nodes_num):
    global _last_kw
    assert int(nodes_num) == N and x.shape == (N, F) and kernel.shape == (F, D)
    kw = 9
    in_maps = _prep(x, kernel, edge_vals, rows, cols, kw)
    while in_maps is None:  # pathological edge distribution: rebuild larger
        kw += 1
        in_maps = _prep(x, kernel, edge_vals, rows, cols, kw)
    _last_kw = kw
    if kw not in _cache:
        _cache[kw] = _build(kw)
    nc = _cache[kw]
    res = run_bass_kernel_spmd(nc, in_maps, core_ids=list(range(NCORES)))
    out = np.concatenate([res.results[c]["out"] for c in range(NCORES)], axis=0)
    return out.astype(np.float32)


# revision 25
# speedup vs baseline: 1.0928x; 1.0928x over previous
"""GCN layer (BN -> dense -> sparse softmax -> gather/scatter -> tanh) on 8
Trainium2 NeuronCores.

Strategy (1D edge parallelism, v2 — on-device one-hot, half-windows):
 - Destination nodes sharded 12500/core; each edge lives on the core owning
   its destination row. Host materializes per-edge-slot SOURCE features plus
   a ones column: xs[slot] = [x[col] | 1] (fp16), laid out per 64-node
   half-window in kw 128-edge chunks.
 - The scatter one-hot is built ON DEVICE (v1 DMA'd it: 58 MB/core of
   zeros): M̃[e, i, c] = (loc[e,c] == i) * exp(v[e,c]), two DVE
   tensor_tensor passes in 2x mode against a host-sent replicated iota.
   Folding exp into M̃ means the 58 MB feature stream goes STRAIGHT from
   DRAM into the PE (v1 spent 240 µs of slow-mode DVE scaling it).
 - Per chunk one PE matmul accumulates A[i, 0:128] += M̃_c^T x[col] and
   A[i, 128] += M̃_c^T 1 = softmax denominator, two 64-node half-windows
   packed into one [128, 129] PSUM tile (partition ranges 0:64 / 64:128).
 - Flush per 128-node pair: A -> fp16, PE-transpose num and den column;
   ps2 = AT^T @ W' + den ⊗ b' (rank-1 bias matmul makes den==0 rows give
   tanh(0) = 0 with no masking); th = Tanh(ps2 * rec) fused on the act
   engine with per-partition scale rec = 1/max(den, eps).
 - BatchNorm folds into the projection: per-core partial sums -> AllReduce
   (the only collective) -> W' = rstd*W, b' = -mean*rstd @ W'.
 - Softmax needs no max subtraction: edge_vals are uniform [0,1).

DMA queues: xs stream on sync; small loads on scalar; stores + collective
staging on gpsimd. Numerics: matmul operands fp16 (PSUM fp32), stats and
softmax denominator fp32.
"""
import sys

sys.path.insert(0, "/opt/trn_rl_repo")

import numpy as np
from contextlib import ExitStack

import concourse.bass as bass
import concourse.bacc as bacc
import concourse.mybir as mybir
import concourse.tile as tile
from concourse.bass_utils import run_bass_kernel_spmd

# problem constants
N = 100000
E = 1600000
F = 128
D = 64
BN_EPS = 1e-3
NCORES = 8
NPC = N // NCORES            # 12500 destination nodes per core
WIN = 64                     # destination nodes per half-window
NW = (NPC + WIN - 1) // WIN  # 196 half-windows per core (last has 20 nodes)
GHW = 6                      # half-windows per processing group

f16, f32 = mybir.dt.float16, mybir.dt.float32

_cache: dict[int, object] = {}
_last_kw: int | None = None


def _groups():
    gs, h = [], 0
    while h < NW:
        g = min(GHW, NW - h)
        gs.append((h, g))
        h += g
    return gs


XBAR = False
LAG_DEFAULT = 16
ABUF = 3
DTPBUF = 1
DUALQ = False

def _build(kw: int, xbar=None, lag=None):
    """Build the SPMD program. kw = max 128-edge chunks per half-window."""
    xbar = XBAR if xbar is None else xbar
    lag = LAG_DEFAULT if lag is None else lag
    nch = NW * kw                    # chunks per core
    gmax = GHW * kw                  # chunks in a full group

    nc = bacc.Bacc(None, target_bir_lowering=False)

    xT = nc.declare_dram_parameter("xT", [F, NPC], f16, isOutput=False)
    w_in = nc.declare_dram_parameter("w_in", [F, D], f32, isOutput=False)
    ident_in = nc.declare_dram_parameter("ident_in", [128, 128], f16, isOutput=False)
    iota_in = nc.declare_dram_parameter("iota_in", [128, WIN * gmax], f16, isOutput=False)
    loc_in = nc.declare_dram_parameter("loc_in", [128, nch], f16, isOutput=False)
    val_in = nc.declare_dram_parameter("val_in", [128, nch], f16, isOutput=False)
    xs_in = nc.declare_dram_parameter("xs_in", [128, nch * (F + 1)], f16, isOutput=False)
    out_p = nc.declare_dram_parameter("out", [NPC, D], f32, isOutput=True)

    with tile.TileContext(nc) as tc:
        with ExitStack() as ctx:
            sb = ctx.enter_context(tc.tile_pool(name="sb", bufs=1))
            pp = ctx.enter_context(tc.tile_pool(name="pp", bufs=1, space="PSUM"))
            dram = ctx.enter_context(tc.tile_pool(name="dram", bufs=1, space="DRAM"))

            # The small phase-1 parameters load FIRST on the sync queue: the
            # one-hot builds (DVE) need loc/iota/exp within ~10 us or the PE
            # starves at the head of the pipeline.
            val_sb = sb.tile([128, nch], f16)
            nc.sync.dma_start(out=val_sb[:], in_=val_in[:])
            loc_sb = sb.tile([128, nch], f16)
            nc.sync.dma_start(out=loc_sb[:], in_=loc_in[:])
            iota_sb = sb.tile([128, WIN, gmax], f16)
            nc.sync.dma_start(out=iota_sb[:], in_=iota_in[:])
            ident_sb = sb.tile([128, 128], f16)
            nc.sync.dma_start(out=ident_sb[:], in_=ident_in[:])
            exp_sb = sb.tile([128, nch], f16)
            nc.scalar.activation(out=exp_sb[:], in_=val_sb[:],
                                 func=mybir.ActivationFunctionType.Exp)

            # ---------------- phase 0: BN stats -> W', bias ----------------
            # xT arrives in 4 slices so the reduce/square pipeline with the
            # DMA; the AllReduce (the long pole before wp) starts ~15us
            # earlier than with a monolithic load. The sum-reduces run on the
            # otherwise-idle pool engine so the DVE queue head stays free for
            # the one-hot builds.
            NSL = 4
            SL = NPC // NSL
            xts = sb.tile([F, NPC], f16)
            parts = sb.tile([F, NSL, 2], f32)
            sq_trash = sb.tile([F, NPC], f16)
            for si in range(NSL):
                sl = slice(si * SL, (si + 1) * SL)
                nc.scalar.dma_start(out=xts[:, sl], in_=xT[:, sl])
                nc.vector.tensor_reduce(
                    out=parts[:, si, 0:1], in_=xts[:, sl],
                    axis=mybir.AxisListType.X, op=mybir.AluOpType.add)
                nc.scalar.activation(
                    out=sq_trash[:, sl], in_=xts[:, sl],
                    func=mybir.ActivationFunctionType.Square,
                    accum_out=parts[:, si, 1:2])
            stats = sb.tile([F, 2], f32)
            nc.vector.tensor_reduce(
                out=stats[:].unsqueeze(2), in_=parts[:].transpose([0, 2, 1]),
                axis=mybir.AxisListType.X, op=mybir.AluOpType.add)

            st_b = dram.tile([F, 2], f32)
            red_b = dram.tile([F, 2], f32)
            nc.gpsimd.dma_start(out=st_b[:], in_=stats[:])
            nc.gpsimd.collective_compute(
                "AllReduce", mybir.AluOpType.add,
                replica_groups=[list(range(NCORES))],
                ins=[st_b[:].opt()], outs=[red_b[:].opt()])
            red = sb.tile([F, 2], f32)
            nc.gpsimd.dma_start(out=red[:], in_=red_b[:])

            mean = sb.tile([F, 1], f32)
            nc.vector.tensor_scalar_mul(out=mean[:], in0=red[:, 0:1], scalar1=1.0 / N)
            ex2 = sb.tile([F, 1], f32)
            nc.vector.tensor_scalar_mul(out=ex2[:], in0=red[:, 1:2], scalar1=1.0 / N)
            msq = sb.tile([F, 1], f32)
            nc.vector.tensor_tensor(out=msq[:], in0=mean[:], in1=mean[:],
                                    op=mybir.AluOpType.mult)
            varep = sb.tile([F, 1], f32)
            nc.vector.tensor_tensor(out=varep[:], in0=ex2[:], in1=msq[:],
                                    op=mybir.AluOpType.subtract)
            nc.vector.tensor_scalar_add(out=varep[:], in0=varep[:], scalar1=BN_EPS)
            sdev = sb.tile([F, 1], f32)
            nc.scalar.activation(out=sdev[:], in_=varep[:],
                                 func=mybir.ActivationFunctionType.Sqrt)
            rstd = sb.tile([F, 1], f32)
            nc.vector.reciprocal(out=rstd[:], in_=sdev[:])

            w_sb = sb.tile([F, D], f32)
            nc.scalar.dma_start(out=w_sb[:], in_=w_in[:])
            wp = sb.tile([F, D], f16)
            nc.vector.tensor_scalar(out=wp[:], in0=w_sb[:], scalar1=rstd[:, 0:1],
                                    scalar2=None, op0=mybir.AluOpType.mult)
            nmr = sb.tile([F, 1], f32)
            nc.vector.tensor_tensor(out=nmr[:], in0=mean[:], in1=rstd[:],
                                    op=mybir.AluOpType.mult)
            nmr16 = sb.tile([F, 1], f16)
            nc.vector.tensor_scalar_mul(out=nmr16[:], in0=nmr[:], scalar1=-1.0)

            # b16 = -mean*rstd @ W' is emitted lazily (right before the first
            # stage-2 drain) so its matmul doesn't sit at the head of the
            # in-order PE queue gating all chunk matmuls on the AllReduce.
            b16_box = []

            def emit_b16():
                b_ps = pp.tile([128, D], f32, tag="init", bufs=1)
                nc.tensor.matmul(out=b_ps[:1, :], lhsT=nmr16[:], rhs=wp[:],
                                 start=True, stop=True)
                b16 = sb.tile([1, D], f16)
                nc.vector.tensor_copy(out=b16[:], in_=b_ps[:1, :])
                b16_box.append(b16)

            # ---------------- phase 1: edges ----------------
            # Stage 2 of the flush (projection ps2 = AT^T @ W' + den*b', tanh,
            # store) depends on the AllReduce-derived wp/b16. Engine queues
            # execute in order, so emitting stage 2 inline would stall every
            # queue behind the collective for ~60 us. Instead stage 1 banks
            # ATs/drow/rec in deep SBUF rings and stage 2 drains with a lag.
            pending = []

            def stage2(ent):
                pr, m, ATs_t, drow_t, rec_t = ent
                ps2 = pp.tile([128, D], f32, tag="ps2", bufs=2)
                nc.tensor.matmul(out=ps2[:], lhsT=ATs_t[:], rhs=wp[:],
                                 start=True, stop=False)
                nc.tensor.matmul(out=ps2[:], lhsT=drow_t[:], rhs=b16_box[0][:],
                                 start=False, stop=True)
                th = sb.tile([128, D], f32, tag="th", bufs=4)
                nc.scalar.activation(
                    out=th[:], in_=ps2[:],
                    func=mybir.ActivationFunctionType.Tanh,
                    scale=rec_t[:, 0:1])
                nc.gpsimd.dma_start(out=out_p[pr * 128:pr * 128 + m, :],
                                    in_=th[:m, :])

            groups = _groups()
            NG = len(groups)
            LAG_G = lag
            for g, (hw0, ghw) in enumerate(groups):
                ch0 = hw0 * kw
                gch = ghw * kw
                # Alternate the 58 MB feature stream across two DMA queues so
                # transfers overlap (one queue tops out ~275 GB/s on HW).
                # Separate tag rings so a queue never blocks on the other's
                # buffer being freed.
                # Dual-queue the feature stream, but keep early groups off the
                # pool queue: the AllReduce occupies it for ~60 us and any xs
                # load queued behind it would starve the PE (FIFO queues).
                use_pool = DUALQ and g % 2 == 1 and g >= 17
                xs = sb.tile([128, gch, F + 1], f16,
                             tag=f"xs{1 if use_pool else 0}", bufs=2)
                qeng = (nc.gpsimd if DUALQ == 'pool' else nc.scalar) if use_pool else nc.sync
                qeng.dma_start(
                    out=xs[:], in_=xs_in[:, ch0 * (F + 1):(ch0 + gch) * (F + 1)])
                mt = sb.tile([128, WIN, gch], f16, tag="mt", bufs=2)
                nc.vector.tensor_tensor(
                    out=mt[:],
                    in0=loc_sb[:, ch0:ch0 + gch].unsqueeze(1).to_broadcast(
                        [128, WIN, gch]),
                    in1=iota_sb[:, :, 0:gch],
                    op=mybir.AluOpType.is_equal)
                nc.vector.tensor_tensor(
                    out=mt[:], in0=mt[:],
                    in1=exp_sb[:, ch0:ch0 + gch].unsqueeze(1).to_broadcast(
                        [128, WIN, gch]),
                    op=mybir.AluOpType.mult)
                for j in range(ghw):
                    h = hw0 + j
                    side = h % 2
                    if side == 0:
                        A = pp.tile([128, F + 1], f32, tag="A", bufs=ABUF)
                    for c in range(kw):
                        lc = j * kw + c
                        nc.tensor.matmul(
                            out=A[side * WIN:(side + 1) * WIN, :],
                            lhsT=mt[:, :, lc], rhs=xs[:, lc, :],
                            start=(c == 0), stop=(c == kw - 1))
                    if side == 1:
                        pr = h // 2
                        m = min(128, NPC - pr * 128)
                        As = sb.tile([128, F + 1], f16, tag="As", bufs=4)
                        nc.scalar.activation(
                            out=As[:], in_=A[:],
                            func=mybir.ActivationFunctionType.Copy)
                        ATs = sb.tile([128, F], f16, tag="ATs", bufs=98)
                        if xbar:
                            nc.scalar.dma_start_transpose(out=ATs[:], in_=As[:, 0:F])
                        else:
                            ATp = pp.tile([128, F], f16, tag="ATp", bufs=1)
                            nc.tensor.transpose(out=ATp[:], in_=As[:, 0:F],
                                                identity=ident_sb[:])
                            nc.scalar.activation(
                                out=ATs[:], in_=ATp[:],
                                func=mybir.ActivationFunctionType.Copy)
                        dtp = pp.tile([1, 128], f16, tag="dtp", bufs=DTPBUF)
                        nc.tensor.transpose(out=dtp[:], in_=As[:, F:F + 1],
                                            identity=ident_sb[:])
                        drow = sb.tile([1, 128], f16, tag="drow", bufs=98)
                        nc.scalar.activation(
                            out=drow[:], in_=dtp[:],
                            func=mybir.ActivationFunctionType.Copy)
                        dmax = sb.tile([128, 1], f32, tag="dmax", bufs=4)
                        nc.vector.tensor_scalar_max(out=dmax[:], in0=A[:, F:F + 1],
                                                    scalar1=1e-30)
                        rec = sb.tile([128, 1], f32, tag="rec", bufs=98)
                        nc.vector.reciprocal(out=rec[:], in_=dmax[:])
                        pending.append((pr, m, ATs, drow, rec))
                if g >= LAG_G and pending:
                    if not b16_box:
                        emit_b16()
                    k = -(-len(pending) // max(1, NG - 2 - g))
                    for ent in pending[:k]:
                        stage2(ent)
                    del pending[:k]
            if pending and not b16_box:
                emit_b16()
            for ent in pending:
                stage2(ent)

    nc.finalize()
    return nc


def _prep(x, w, edge_vals, rows, cols, kw):
    """Host-side shard/layout construction. Returns in_maps or None if kw
    is too small for this edge distribution."""
    nch = NW * kw
    gmax = GHW * kw

    order = np.argsort(rows, kind="stable")
    rs = rows[order].astype(np.int64)
    cs = cols[order].astype(np.int64)
    vs = edge_vals[order]

    core = rs // NPC
    lic = rs % NPC
    hwin = lic // WIN
    loc = lic % WIN

    run = core * NW + hwin               # global half-window id, monotone in rs
    nruns = NCORES * NW
    counts = np.bincount(run, minlength=nruns)
    if counts.max() > kw * 128:
        return None
    starts = np.zeros(nruns, np.int64)
    np.cumsum(counts[:-1], out=starts[1:])
    pos = np.arange(len(run)) - starts[run]

    chunk = hwin * kw + pos // 128       # chunk index within the core
    e_part = pos % 128

    locf = np.full((NCORES, 128, nch), -1.0, np.float16)
    valf = np.full((NCORES, 128, nch), -100.0, np.float16)
    colf = np.zeros((NCORES, 128, nch), np.int64)
    locf[core, e_part, chunk] = loc.astype(np.float16)
    valf[core, e_part, chunk] = vs
    colf[core, e_part, chunk] = cs

    x16 = x.astype(np.float16)
    ident = np.eye(128, dtype=np.float16)
    iota = np.ascontiguousarray(np.broadcast_to(
        np.arange(WIN, dtype=np.float16)[None, :, None],
        (128, WIN, gmax))).reshape(128, WIN * gmax)
    w32 = np.ascontiguousarray(w.astype(np.float32))
    in_maps = []
    for c in range(NCORES):
        xs = np.empty((128, nch, F + 1), np.float16)
        xs[:, :, 0:F] = x16[colf[c]]
        xs[:, :, F] = 1.0
        xsh = np.ascontiguousarray(x16[c * NPC:(c + 1) * NPC, :].T)
        in_maps.append({
            "xT": xsh,
            "w_in": w32,
            "ident_in": ident,
            "iota_in": iota,
            "loc_in": np.ascontiguousarray(locf[c]),
            "val_in": np.ascontiguousarray(valf[c]),
            "xs_in": xs.reshape(128, nch * (F + 1)),
        })
    return in_maps


def kernel(x, kernel, edge_vals, rows, cols, nodes_num):
    global _last_kw
    assert int(nodes_num) == N and x.shape == (N, F) and kernel.shape == (F, D)
    kw = 9
    in_maps = _prep(x, kernel, edge_vals, rows, cols, kw)
    while in_maps is None:  # pathological edge distribution: rebuild larger
        kw += 1
        in_maps = _prep(x, kernel, edge_vals, rows, cols, kw)
    _last_kw = kw
    if kw not in _cache:
        _cache[kw] = _build(kw)
    nc = _cache[kw]
    res = run_bass_kernel_spmd(nc, in_maps, core_ids=list(range(NCORES)))
    out = np.concatenate([res.results[c]["out"] for c in range(NCORES)], axis=0)
    return out.astype(np.float32)


# revision 28
# speedup vs baseline: 1.2302x; 1.1258x over previous
"""GCN layer (BN -> dense -> sparse softmax -> gather/scatter -> tanh) on 8
Trainium2 NeuronCores.

Strategy (1D edge parallelism, v2 — on-device one-hot, half-windows):
 - Destination nodes sharded 12500/core; each edge lives on the core owning
   its destination row. Host materializes per-edge-slot SOURCE features plus
   a ones column: xs[slot] = [x[col] | 1] (fp16), laid out per 64-node
   half-window in kw 128-edge chunks.
 - The scatter one-hot is built ON DEVICE (v1 DMA'd it: 58 MB/core of
   zeros): M̃[e, i, c] = (loc[e,c] == i) * exp(v[e,c]), two DVE
   tensor_tensor passes in 2x mode against a host-sent replicated iota.
   Folding exp into M̃ means the 58 MB feature stream goes STRAIGHT from
   DRAM into the PE (v1 spent 240 µs of slow-mode DVE scaling it).
 - Per chunk one PE matmul accumulates A[i, 0:128] += M̃_c^T x[col] and
   A[i, 128] += M̃_c^T 1 = softmax denominator, two 64-node half-windows
   packed into one [128, 129] PSUM tile (partition ranges 0:64 / 64:128).
 - Flush per 128-node pair: A -> fp16, PE-transpose num and den column;
   ps2 = AT^T @ W' + den ⊗ b' (rank-1 bias matmul makes den==0 rows give
   tanh(0) = 0 with no masking); th = Tanh(ps2 * rec) fused on the act
   engine with per-partition scale rec = 1/max(den, eps).
 - BatchNorm folds into the projection: per-core partial sums -> AllReduce
   (the only collective) -> W' = rstd*W, b' = -mean*rstd @ W'.
 - Softmax needs no max subtraction: edge_vals are uniform [0,1).

DMA queues: xs stream on sync; small loads on scalar; stores + collective
staging on gpsimd. Numerics: matmul operands fp16 (PSUM fp32), stats and
softmax denominator fp32.
"""
import sys

sys.path.insert(0, "/opt/trn_rl_repo")

import numpy as np
from contextlib import ExitStack

import concourse.bass as bass
import concourse.bacc as bacc
import concourse.mybir as mybir
import concourse.tile as tile
from concourse.bass_utils import run_bass_kernel_spmd

# problem constants
N = 100000
E = 1600000
F = 128
D = 64
BN_EPS = 1e-3
NCORES = 8
NPC = N // NCORES            # 12500 destination nodes per core
WIN = 64                     # destination nodes per half-window
NW = (NPC + WIN - 1) // WIN  # 196 half-windows per core (last has 20 nodes)
GHW = 6                      # half-windows per processing group

f16, f32 = mybir.dt.float16, mybir.dt.float32

_cache: dict[int, object] = {}
_last_kw: int | None = None


def _groups():
    gs, h = [], 0
    while h < NW:
        g = min(GHW, NW - h)
        gs.append((h, g))
        h += g
    return gs


XBAR = False
LAG_DEFAULT = 16
ABUF = 3
DTPBUF = 1
DUALQ = False

def _build(kw: int, xbar=None, lag=None):
    """Build the SPMD program. kw = max 128-edge chunks per half-window."""
    xbar = XBAR if xbar is None else xbar
    lag = LAG_DEFAULT if lag is None else lag
    nch = NW * kw                    # chunks per core
    gmax = GHW * kw                  # chunks in a full group

    nc = bacc.Bacc(None, target_bir_lowering=False)

    xT = nc.declare_dram_parameter("xT", [F, NPC], f16, isOutput=False)
    w_in = nc.declare_dram_parameter("w_in", [F, D], f32, isOutput=False)
    ident_in = nc.declare_dram_parameter("ident_in", [128, 128], f16, isOutput=False)
    iota_in = nc.declare_dram_parameter("iota_in", [128, WIN * gmax], f16, isOutput=False)
    loc_in = nc.declare_dram_parameter("loc_in", [128, nch], f16, isOutput=False)
    val_in = nc.declare_dram_parameter("val_in", [128, nch], f16, isOutput=False)
    xs_in = nc.declare_dram_parameter("xs_in", [128, nch * (F + 1)], f16, isOutput=False)
    out_p = nc.declare_dram_parameter("out", [NPC, D], f16, isOutput=True)

    with tile.TileContext(nc) as tc:
        with ExitStack() as ctx:
            sb = ctx.enter_context(tc.tile_pool(name="sb", bufs=1))
            pp = ctx.enter_context(tc.tile_pool(name="pp", bufs=1, space="PSUM"))
            dram = ctx.enter_context(tc.tile_pool(name="dram", bufs=1, space="DRAM"))

            # The small phase-1 parameters load FIRST on the sync queue: the
            # one-hot builds (DVE) need loc/iota/exp within ~10 us or the PE
            # starves at the head of the pipeline.
            val_sb = sb.tile([128, nch], f16)
            nc.sync.dma_start(out=val_sb[:], in_=val_in[:])
            loc_sb = sb.tile([128, nch], f16)
            nc.sync.dma_start(out=loc_sb[:], in_=loc_in[:])
            iota_sb = sb.tile([128, WIN, gmax], f16)
            nc.sync.dma_start(out=iota_sb[:], in_=iota_in[:])
            ident_sb = sb.tile([128, 128], f16)
            nc.sync.dma_start(out=ident_sb[:], in_=ident_in[:])
            exp_sb = sb.tile([128, nch], f16)
            nc.scalar.activation(out=exp_sb[:], in_=val_sb[:],
                                 func=mybir.ActivationFunctionType.Exp)

            # ---------------- phase 0: BN stats -> W', bias ----------------
            # xT arrives in 4 slices so the reduce/square pipeline with the
            # DMA; the AllReduce (the long pole before wp) starts ~15us
            # earlier than with a monolithic load. The sum-reduces run on the
            # otherwise-idle pool engine so the DVE queue head stays free for
            # the one-hot builds.
            NSL = 4
            SL = NPC // NSL
            xts = sb.tile([F, NPC], f16)
            parts = sb.tile([F, NSL, 2], f32)
            sq_trash = sb.tile([F, NPC], f16)
            for si in range(NSL):
                sl = slice(si * SL, (si + 1) * SL)
                nc.scalar.dma_start(out=xts[:, sl], in_=xT[:, sl])
                nc.vector.tensor_reduce(
                    out=parts[:, si, 0:1], in_=xts[:, sl],
                    axis=mybir.AxisListType.X, op=mybir.AluOpType.add)
                nc.scalar.activation(
                    out=sq_trash[:, sl], in_=xts[:, sl],
                    func=mybir.ActivationFunctionType.Square,
                    accum_out=parts[:, si, 1:2])
            stats = sb.tile([F, 2], f32)
            nc.vector.tensor_reduce(
                out=stats[:].unsqueeze(2), in_=parts[:].transpose([0, 2, 1]),
                axis=mybir.AxisListType.X, op=mybir.AluOpType.add)

            st_b = dram.tile([F, 2], f32)
            red_b = dram.tile([F, 2], f32)
            nc.gpsimd.dma_start(out=st_b[:], in_=stats[:])
            nc.gpsimd.collective_compute(
                "AllReduce", mybir.AluOpType.add,
                replica_groups=[list(range(NCORES))],
                ins=[st_b[:].opt()], outs=[red_b[:].opt()])
            red = sb.tile([F, 2], f32)
            nc.gpsimd.dma_start(out=red[:], in_=red_b[:])

            mean = sb.tile([F, 1], f32)
            nc.vector.tensor_scalar_mul(out=mean[:], in0=red[:, 0:1], scalar1=1.0 / N)
            ex2 = sb.tile([F, 1], f32)
            nc.vector.tensor_scalar_mul(out=ex2[:], in0=red[:, 1:2], scalar1=1.0 / N)
            msq = sb.tile([F, 1], f32)
            nc.vector.tensor_tensor(out=msq[:], in0=mean[:], in1=mean[:],
                                    op=mybir.AluOpType.mult)
            varep = sb.tile([F, 1], f32)
            nc.vector.tensor_tensor(out=varep[:], in0=ex2[:], in1=msq[:],
                                    op=mybir.AluOpType.subtract)
            nc.vector.tensor_scalar_add(out=varep[:], in0=varep[:], scalar1=BN_EPS)
            sdev = sb.tile([F, 1], f32)
            nc.scalar.activation(out=sdev[:], in_=varep[:],
                                 func=mybir.ActivationFunctionType.Sqrt)
            rstd = sb.tile([F, 1], f32)
            nc.vector.reciprocal(out=rstd[:], in_=sdev[:])

            w_sb = sb.tile([F, D], f32)
            nc.scalar.dma_start(out=w_sb[:], in_=w_in[:])
            wp = sb.tile([F, D], f16)
            nc.vector.tensor_scalar(out=wp[:], in0=w_sb[:], scalar1=rstd[:, 0:1],
                                    scalar2=None, op0=mybir.AluOpType.mult)
            nmr = sb.tile([F, 1], f32)
            nc.vector.tensor_tensor(out=nmr[:], in0=mean[:], in1=rstd[:],
                                    op=mybir.AluOpType.mult)
            nmr16 = sb.tile([F, 1], f16)
            nc.vector.tensor_scalar_mul(out=nmr16[:], in0=nmr[:], scalar1=-1.0)

            # b16 = -mean*rstd @ W' is emitted lazily (right before the first
            # stage-2 drain) so its matmul doesn't sit at the head of the
            # in-order PE queue gating all chunk matmuls on the AllReduce.
            b16_box = []

            def emit_b16():
                b_ps = pp.tile([128, D], f32, tag="init", bufs=1)
                nc.tensor.matmul(out=b_ps[:1, :], lhsT=nmr16[:], rhs=wp[:],
                                 start=True, stop=True)
                b16 = sb.tile([1, D], f16)
                nc.vector.tensor_copy(out=b16[:], in_=b_ps[:1, :])
                b16_box.append(b16)

            # ---------------- phase 1: edges ----------------
            # Stage 2 of the flush (projection ps2 = AT^T @ W' + den*b', tanh,
            # store) depends on the AllReduce-derived wp/b16. Engine queues
            # execute in order, so emitting stage 2 inline would stall every
            # queue behind the collective for ~60 us. Instead stage 1 banks
            # ATs/drow/rec in deep SBUF rings and stage 2 drains with a lag.
            pending = []

            # Pairs drain two-at-a-time sharing one th tile and one store DMA
            # (halves the per-store DGE overhead on the pool queue).
            th_box = []

            def stage2(ent):
                pr, m, ATs_t, drow_t, rec_t = ent
                ps2 = pp.tile([128, D], f32, tag="ps2", bufs=2)
                nc.tensor.matmul(out=ps2[:], lhsT=ATs_t[:], rhs=wp[:],
                                 start=True, stop=False)
                nc.tensor.matmul(out=ps2[:], lhsT=drow_t[:], rhs=b16_box[0][:],
                                 start=False, stop=True)
                slot = pr % 2
                if slot == 0:
                    th = sb.tile([128, 2, D], f16, tag="th", bufs=4)
                    th_box.append(th)
                th = th_box[0]
                nc.scalar.activation(
                    out=th[:, slot, :], in_=ps2[:],
                    func=mybir.ActivationFunctionType.Tanh,
                    scale=rec_t[:, 0:1])
                if slot == 1 and m == 128:
                    pr0 = pr - 1
                    dst = out_p[pr0 * 128:pr0 * 128 + 256, :].rearrange(
                        "(k p) d -> p k d", k=2)
                    nc.gpsimd.dma_start(out=dst, in_=th[:])
                    th_box.clear()
                elif slot == 1:  # short final pair: two separate stores
                    nc.gpsimd.dma_start(
                        out=out_p[(pr - 1) * 128:pr * 128, :], in_=th[:, 0, :])
                    nc.gpsimd.dma_start(out=out_p[pr * 128:pr * 128 + m, :],
                                        in_=th[:m, 1, :])
                    th_box.clear()

            groups = _groups()
            NG = len(groups)
            LAG_G = lag
            for g, (hw0, ghw) in enumerate(groups):
                ch0 = hw0 * kw
                gch = ghw * kw
                # Alternate the 58 MB feature stream across two DMA queues so
                # transfers overlap (one queue tops out ~275 GB/s on HW).
                # Separate tag rings so a queue never blocks on the other's
                # buffer being freed.
                # Dual-queue the feature stream, but keep early groups off the
                # pool queue: the AllReduce occupies it for ~60 us and any xs
                # load queued behind it would starve the PE (FIFO queues).
                use_pool = DUALQ and g % 2 == 1 and g >= 17
                xs = sb.tile([128, gch, F + 1], f16,
                             tag=f"xs{1 if use_pool else 0}", bufs=2)
                qeng = (nc.gpsimd if DUALQ == 'pool' else nc.scalar) if use_pool else nc.sync
                qeng.dma_start(
                    out=xs[:], in_=xs_in[:, ch0 * (F + 1):(ch0 + gch) * (F + 1)])
                mt = sb.tile([128, WIN, gch], f16, tag="mt", bufs=2)
                nc.vector.tensor_tensor(
                    out=mt[:],
                    in0=loc_sb[:, ch0:ch0 + gch].unsqueeze(1).to_broadcast(
                        [128, WIN, gch]),
                    in1=iota_sb[:, :, 0:gch],
                    op=mybir.AluOpType.is_equal)
                nc.vector.tensor_tensor(
                    out=mt[:], in0=mt[:],
                    in1=exp_sb[:, ch0:ch0 + gch].unsqueeze(1).to_broadcast(
                        [128, WIN, gch]),
                    op=mybir.AluOpType.mult)
                for j in range(ghw):
                    h = hw0 + j
                    side = h % 2
                    if side == 0:
                        A = pp.tile([128, F + 1], f32, tag="A", bufs=ABUF)
                    for c in range(kw):
                        lc = j * kw + c
                        nc.tensor.matmul(
                            out=A[side * WIN:(side + 1) * WIN, :],
                            lhsT=mt[:, :, lc], rhs=xs[:, lc, :],
                            start=(c == 0), stop=(c == kw - 1))
                    if side == 1:
                        pr = h // 2
                        m = min(128, NPC - pr * 128)
                        As = sb.tile([128, F + 1], f16, tag="As", bufs=4)
                        nc.scalar.activation(
                            out=As[:], in_=A[:],
                            func=mybir.ActivationFunctionType.Copy)
                        ATs = sb.tile([128, F], f16, tag="ATs", bufs=98)
                        if xbar:
                            nc.scalar.dma_start_transpose(out=ATs[:], in_=As[:, 0:F])
                        else:
                            ATp = pp.tile([128, F], f16, tag="ATp", bufs=1)
                            nc.tensor.transpose(out=ATp[:], in_=As[:, 0:F],
                                                identity=ident_sb[:])
                            nc.scalar.activation(
                                out=ATs[:], in_=ATp[:],
                                func=mybir.ActivationFunctionType.Copy)
                        dtp = pp.tile([1, 128], f16, tag="dtp", bufs=DTPBUF)
                        nc.tensor.transpose(out=dtp[:], in_=As[:, F:F + 1],
                                            identity=ident_sb[:])
                        drow = sb.tile([1, 128], f16, tag="drow", bufs=98)
                        nc.scalar.activation(
                            out=drow[:], in_=dtp[:],
                            func=mybir.ActivationFunctionType.Copy)
                        dmax = sb.tile([128, 1], f32, tag="dmax", bufs=4)
                        nc.vector.tensor_scalar_max(out=dmax[:], in0=A[:, F:F + 1],
                                                    scalar1=1e-30)
                        rec = sb.tile([128, 1], f32, tag="rec", bufs=98)
                        nc.vector.reciprocal(out=rec[:], in_=dmax[:])
                        pending.append((pr, m, ATs, drow, rec))
                if g >= LAG_G and pending:
                    if not b16_box:
                        emit_b16()
                    k = -(-len(pending) // max(1, NG - 2 - g))
                    for ent in pending[:k]:
                        stage2(ent)
                    del pending[:k]
            if pending and not b16_box:
                emit_b16()
            for ent in pending:
                stage2(ent)

    nc.finalize()
    return nc


def _prep(x, w, edge_vals, rows, cols, kw):
    """Host-side shard/layout construction. Returns in_maps or None if kw
    is too small for this edge distribution."""
    nch = NW * kw
    gmax = GHW * kw

    order = np.argsort(rows, kind="stable")
    rs = rows[order].astype(np.int64)
    cs = cols[order].astype(np.int64)
    vs = edge_vals[order]

    core = rs // NPC
    lic = rs % NPC
    hwin = lic // WIN
    loc = lic % WIN

    run = core * NW + hwin               # global half-window id, monotone in rs
    nruns = NCORES * NW
    counts = np.bincount(run, minlength=nruns)
    if counts.max() > kw * 128:
        return None
    starts = np.zeros(nruns, np.int64)
    np.cumsum(counts[:-1], out=starts[1:])
    pos = np.arange(len(run)) - starts[run]

    chunk = hwin * kw + pos // 128       # chunk index within the core
    e_part = pos % 128

    locf = np.full((NCORES, 128, nch), -1.0, np.float16)
    valf = np.full((NCORES, 128, nch), -100.0, np.float16)
    colf = np.zeros((NCORES, 128, nch), np.int64)
    locf[core, e_part, chunk] = loc.astype(np.float16)
    valf[core, e_part, chunk] = vs
    colf[core, e_part, chunk] = cs

    x16 = x.astype(np.float16)
    ident = np.eye(128, dtype=np.float16)
    iota = np.ascontiguousarray(np.broadcast_to(
        np.arange(WIN, dtype=np.float16)[None, :, None],
        (128, WIN, gmax))).reshape(128, WIN * gmax)
    w32 = np.ascontiguousarray(w.astype(np.float32))
    in_maps = []
    for c in range(NCORES):
        xs = np.empty((128, nch, F + 1), np.float16)
        xs[:, :, 0:F] = x16[colf[c]]
        xs[:, :, F] = 1.0
        xsh = np.ascontiguousarray(x16[c * NPC:(c + 1) * NPC, :].T)
        in_maps.append({
            "xT": xsh,
            "w_in": w32,
            "ident_in": ident,
            "iota_in": iota,
            "loc_in": np.ascontiguousarray(locf[c]),
            "val_in": np.ascontiguousarray(valf[c]),
            "xs_in": xs.reshape(128, nch * (F + 1)),
        })
    return in_maps


def kernel(x, kernel, edge_vals, rows, cols, nodes_num):
    global _last_kw
    assert int(nodes_num) == N and x.shape == (N, F) and kernel.shape == (F, D)
    kw = 9
    in_maps = _prep(x, kernel, edge_vals, rows, cols, kw)
    while in_maps is None:  # pathological edge distribution: rebuild larger
        kw += 1
        in_maps = _prep(x, kernel, edge_vals, rows, cols, kw)
    _last_kw = kw
    if kw not in _cache:
        _cache[kw] = _build(kw)
    nc = _cache[kw]
    res = run_bass_kernel_spmd(nc, in_maps, core_ids=list(range(NCORES)))
    out = np.concatenate([res.results[c]["out"] for c in range(NCORES)], axis=0)
    return out.astype(np.float32)


# revision 31
# speedup vs baseline: 1.2517x; 1.0174x over previous
"""GCN layer (BN -> dense -> sparse softmax -> gather/scatter -> tanh) on 8
Trainium2 NeuronCores.

Strategy (1D edge parallelism, v2 — on-device one-hot, half-windows):
 - Destination nodes sharded 12500/core; each edge lives on the core owning
   its destination row. Host materializes per-edge-slot SOURCE features plus
   a ones column: xs[slot] = [x[col] | 1] (fp16), laid out per 64-node
   half-window in kw 128-edge chunks.
 - The scatter one-hot is built ON DEVICE (v1 DMA'd it: 58 MB/core of
   zeros): M̃[e, i, c] = (loc[e,c] == i) * exp(v[e,c]), two DVE
   tensor_tensor passes in 2x mode against a host-sent replicated iota.
   Folding exp into M̃ means the 58 MB feature stream goes STRAIGHT from
   DRAM into the PE (v1 spent 240 µs of slow-mode DVE scaling it).
 - Per chunk one PE matmul accumulates A[i, 0:128] += M̃_c^T x[col] and
   A[i, 128] += M̃_c^T 1 = softmax denominator, two 64-node half-windows
   packed into one [128, 129] PSUM tile (partition ranges 0:64 / 64:128).
 - Flush per 128-node pair: A -> fp16, PE-transpose num and den column;
   ps2 = AT^T @ W' + den ⊗ b' (rank-1 bias matmul makes den==0 rows give
   tanh(0) = 0 with no masking); th = Tanh(ps2 * rec) fused on the act
   engine with per-partition scale rec = 1/max(den, eps).
 - BatchNorm folds into the projection: per-core partial sums -> AllReduce
   (the only collective) -> W' = rstd*W, b' = -mean*rstd @ W'.
 - Softmax needs no max subtraction: edge_vals are uniform [0,1).

DMA queues: xs stream on sync; small loads on scalar; stores + collective
staging on gpsimd. Numerics: matmul operands fp16 (PSUM fp32), stats and
softmax denominator fp32.
"""
import sys

sys.path.insert(0, "/opt/trn_rl_repo")

import numpy as np
from contextlib import ExitStack

import concourse.bass as bass
import concourse.bacc as bacc
import concourse.mybir as mybir
import concourse.tile as tile
from concourse.bass_utils import run_bass_kernel_spmd

# problem constants
N = 100000
E = 1600000
F = 128
D = 64
BN_EPS = 1e-3
NCORES = 8
NPC = N // NCORES            # 12500 destination nodes per core
WIN = 64                     # destination nodes per half-window
NW = (NPC + WIN - 1) // WIN  # 196 half-windows per core (last has 20 nodes)
GHW = 6                      # half-windows per processing group

f16, f32 = mybir.dt.float16, mybir.dt.float32

_cache: dict[int, object] = {}
_last_kw: int | None = None


def _groups():
    gs, h = [], 0
    while h < NW:
        g = min(GHW, NW - h)
        gs.append((h, g))
        h += g
    return gs


XBAR = False
LAG_DEFAULT = 18
ABUF = 3
DTPBUF = 1
DUALQ = False

def _build(kw: int, xbar=None, lag=None):
    """Build the SPMD program. kw = max 128-edge chunks per half-window."""
    xbar = XBAR if xbar is None else xbar
    lag = LAG_DEFAULT if lag is None else lag
    nch = NW * kw                    # chunks per core
    gmax = GHW * kw                  # chunks in a full group

    nc = bacc.Bacc(None, target_bir_lowering=False)

    xT = nc.declare_dram_parameter("xT", [F, NPC], f16, isOutput=False)
    w_in = nc.declare_dram_parameter("w_in", [F, D], f32, isOutput=False)
    ident_in = nc.declare_dram_parameter("ident_in", [128, 128], f16, isOutput=False)
    iota_in = nc.declare_dram_parameter("iota_in", [128, WIN * gmax], f16, isOutput=False)
    loc_in = nc.declare_dram_parameter("loc_in", [128, nch], f16, isOutput=False)
    val_in = nc.declare_dram_parameter("val_in", [128, nch], f16, isOutput=False)
    xs_in = nc.declare_dram_parameter("xs_in", [128, nch * (F + 1)], f16, isOutput=False)
    out_p = nc.declare_dram_parameter("out", [NPC, D], f16, isOutput=True)

    with tile.TileContext(nc) as tc:
        with ExitStack() as ctx:
            sb = ctx.enter_context(tc.tile_pool(name="sb", bufs=1))
            pp = ctx.enter_context(tc.tile_pool(name="pp", bufs=1, space="PSUM"))
            dram = ctx.enter_context(tc.tile_pool(name="dram", bufs=1, space="DRAM"))

            # The small phase-1 parameters load FIRST on the sync queue: the
            # one-hot builds (DVE) need loc/iota/exp within ~10 us or the PE
            # starves at the head of the pipeline.
            val_sb = sb.tile([128, nch], f16)
            nc.sync.dma_start(out=val_sb[:], in_=val_in[:])
            loc_sb = sb.tile([128, nch], f16)
            nc.sync.dma_start(out=loc_sb[:], in_=loc_in[:])
            iota_sb = sb.tile([128, WIN, gmax], f16)
            nc.sync.dma_start(out=iota_sb[:], in_=iota_in[:])
            ident_sb = sb.tile([128, 128], f16)
            nc.sync.dma_start(out=ident_sb[:], in_=ident_in[:])
            exp_sb = sb.tile([128, nch], f16)
            nc.scalar.activation(out=exp_sb[:], in_=val_sb[:],
                                 func=mybir.ActivationFunctionType.Exp)

            # ---------------- phase 0: BN stats -> W', bias ----------------
            # xT arrives in 4 slices so the reduce/square pipeline with the
            # DMA; the AllReduce (the long pole before wp) starts ~15us
            # earlier than with a monolithic load. The sum-reduces run on the
            # otherwise-idle pool engine so the DVE queue head stays free for
            # the one-hot builds.
            NSL = 4
            SL = NPC // NSL
            xts = sb.tile([F, NPC], f16)
            parts = sb.tile([F, NSL, 2], f32)
            sq_trash = sb.tile([F, NPC], f16)
            for si in range(NSL):
                sl = slice(si * SL, (si + 1) * SL)
                nc.scalar.dma_start(out=xts[:, sl], in_=xT[:, sl])
                nc.vector.tensor_reduce(
                    out=parts[:, si, 0:1], in_=xts[:, sl],
                    axis=mybir.AxisListType.X, op=mybir.AluOpType.add)
                nc.scalar.activation(
                    out=sq_trash[:, sl], in_=xts[:, sl],
                    func=mybir.ActivationFunctionType.Square,
                    accum_out=parts[:, si, 1:2])
            stats = sb.tile([F, 2], f32)
            nc.vector.tensor_reduce(
                out=stats[:].unsqueeze(2), in_=parts[:].transpose([0, 2, 1]),
                axis=mybir.AxisListType.X, op=mybir.AluOpType.add)

            st_b = dram.tile([F, 2], f32)
            red_b = dram.tile([F, 2], f32)
            nc.gpsimd.dma_start(out=st_b[:], in_=stats[:])
            nc.gpsimd.collective_compute(
                "AllReduce", mybir.AluOpType.add,
                replica_groups=[list(range(NCORES))],
                ins=[st_b[:].opt()], outs=[red_b[:].opt()])
            red = sb.tile([F, 2], f32)
            nc.gpsimd.dma_start(out=red[:], in_=red_b[:])

            mean = sb.tile([F, 1], f32)
            nc.vector.tensor_scalar_mul(out=mean[:], in0=red[:, 0:1], scalar1=1.0 / N)
            ex2 = sb.tile([F, 1], f32)
            nc.vector.tensor_scalar_mul(out=ex2[:], in0=red[:, 1:2], scalar1=1.0 / N)
            msq = sb.tile([F, 1], f32)
            nc.vector.tensor_tensor(out=msq[:], in0=mean[:], in1=mean[:],
                                    op=mybir.AluOpType.mult)
            varep = sb.tile([F, 1], f32)
            nc.vector.tensor_tensor(out=varep[:], in0=ex2[:], in1=msq[:],
                                    op=mybir.AluOpType.subtract)
            nc.vector.tensor_scalar_add(out=varep[:], in0=varep[:], scalar1=BN_EPS)
            sdev = sb.tile([F, 1], f32)
            nc.scalar.activation(out=sdev[:], in_=varep[:],
                                 func=mybir.ActivationFunctionType.Sqrt)
            rstd = sb.tile([F, 1], f32)
            nc.vector.reciprocal(out=rstd[:], in_=sdev[:])

            w_sb = sb.tile([F, D], f32)
            nc.scalar.dma_start(out=w_sb[:], in_=w_in[:])
            wp = sb.tile([F, D], f16)
            nc.vector.tensor_scalar(out=wp[:], in0=w_sb[:], scalar1=rstd[:, 0:1],
                                    scalar2=None, op0=mybir.AluOpType.mult)
            nmr = sb.tile([F, 1], f32)
            nc.vector.tensor_tensor(out=nmr[:], in0=mean[:], in1=rstd[:],
                                    op=mybir.AluOpType.mult)
            nmr16 = sb.tile([F, 1], f16)
            nc.vector.tensor_scalar_mul(out=nmr16[:], in0=nmr[:], scalar1=-1.0)

            # b16 = -mean*rstd @ W' is emitted lazily (right before the first
            # stage-2 drain) so its matmul doesn't sit at the head of the
            # in-order PE queue gating all chunk matmuls on the AllReduce.
            b16_box = []

            def emit_b16():
                b_ps = pp.tile([128, D], f32, tag="init", bufs=1)
                nc.tensor.matmul(out=b_ps[:1, :], lhsT=nmr16[:], rhs=wp[:],
                                 start=True, stop=True)
                b16 = sb.tile([1, D], f16)
                nc.vector.tensor_copy(out=b16[:], in_=b_ps[:1, :])
                b16_box.append(b16)

            # ---------------- phase 1: edges ----------------
            # Stage 2 of the flush (projection ps2 = AT^T @ W' + den*b', tanh,
            # store) depends on the AllReduce-derived wp/b16. Engine queues
            # execute in order, so emitting stage 2 inline would stall every
            # queue behind the collective for ~60 us. Instead stage 1 banks
            # ATs/drow/rec in deep SBUF rings and stage 2 drains with a lag.
            pending = []

            # Pairs drain two-at-a-time sharing one th tile and one store DMA
            # (halves the per-store DGE overhead on the pool queue).
            th_box = []

            def stage2(ent):
                pr, m, ATs_t, drow_t, rec_t = ent
                ps2 = pp.tile([128, D], f32, tag="ps2", bufs=2)
                nc.tensor.matmul(out=ps2[:], lhsT=ATs_t[:], rhs=wp[:],
                                 start=True, stop=False)
                nc.tensor.matmul(out=ps2[:], lhsT=drow_t[:], rhs=b16_box[0][:],
                                 start=False, stop=True)
                slot = pr % 2
                if slot == 0:
                    th = sb.tile([128, 2, D], f16, tag="th", bufs=4)
                    th_box.append(th)
                th = th_box[0]
                nc.scalar.activation(
                    out=th[:, slot, :], in_=ps2[:],
                    func=mybir.ActivationFunctionType.Tanh,
                    scale=rec_t[:, 0:1])
                if slot == 1 and m == 128:
                    pr0 = pr - 1
                    dst = out_p[pr0 * 128:pr0 * 128 + 256, :].rearrange(
                        "(k p) d -> p k d", k=2)
                    nc.gpsimd.dma_start(out=dst, in_=th[:])
                    th_box.clear()
                elif slot == 1:  # short final pair: two separate stores
                    nc.gpsimd.dma_start(
                        out=out_p[(pr - 1) * 128:pr * 128, :], in_=th[:, 0, :])
                    nc.gpsimd.dma_start(out=out_p[pr * 128:pr * 128 + m, :],
                                        in_=th[:m, 1, :])
                    th_box.clear()

            groups = _groups()
            NG = len(groups)
            LAG_G = lag
            for g, (hw0, ghw) in enumerate(groups):
                ch0 = hw0 * kw
                gch = ghw * kw
                # Alternate the 58 MB feature stream across two DMA queues so
                # transfers overlap (one queue tops out ~275 GB/s on HW).
                # Separate tag rings so a queue never blocks on the other's
                # buffer being freed.
                # Dual-queue the feature stream, but keep early groups off the
                # pool queue: the AllReduce occupies it for ~60 us and any xs
                # load queued behind it would starve the PE (FIFO queues).
                use_pool = DUALQ and g % 2 == 1 and g >= 17
                xs = sb.tile([128, gch, F + 1], f16,
                             tag=f"xs{1 if use_pool else 0}", bufs=2)
                qeng = (nc.gpsimd if DUALQ == 'pool' else nc.scalar) if use_pool else nc.sync
                qeng.dma_start(
                    out=xs[:], in_=xs_in[:, ch0 * (F + 1):(ch0 + gch) * (F + 1)])
                # One-hot build alternates between DVE and the pool engine so
                # the PE never waits on a single overloaded vector queue.
                beng = nc.vector
                mt = sb.tile([128, WIN, gch], f16, tag="mt", bufs=2)
                beng.tensor_tensor(
                    out=mt[:],
                    in0=loc_sb[:, ch0:ch0 + gch].unsqueeze(1).to_broadcast(
                        [128, WIN, gch]),
                    in1=iota_sb[:, :, 0:gch],
                    op=mybir.AluOpType.is_equal)
                beng.tensor_tensor(
                    out=mt[:], in0=mt[:],
                    in1=exp_sb[:, ch0:ch0 + gch].unsqueeze(1).to_broadcast(
                        [128, WIN, gch]),
                    op=mybir.AluOpType.mult)
                for j in range(ghw):
                    h = hw0 + j
                    side = h % 2
                    if side == 0:
                        A = pp.tile([128, F + 1], f32, tag="A", bufs=ABUF)
                    for c in range(kw):
                        lc = j * kw + c
                        nc.tensor.matmul(
                            out=A[side * WIN:(side + 1) * WIN, :],
                            lhsT=mt[:, :, lc], rhs=xs[:, lc, :],
                            start=(c == 0), stop=(c == kw - 1))
                    if side == 1:
                        pr = h // 2
                        m = min(128, NPC - pr * 128)
                        As = sb.tile([128, F + 1], f16, tag="As", bufs=4)
                        nc.scalar.activation(
                            out=As[:], in_=A[:],
                            func=mybir.ActivationFunctionType.Copy)
                        ATs = sb.tile([128, F], f16, tag="ATs", bufs=98)
                        if xbar:
                            nc.scalar.dma_start_transpose(out=ATs[:], in_=As[:, 0:F])
                        else:
                            ATp = pp.tile([128, F], f16, tag="ATp", bufs=1)
                            nc.tensor.transpose(out=ATp[:], in_=As[:, 0:F],
                                                identity=ident_sb[:])
                            nc.scalar.activation(
                                out=ATs[:], in_=ATp[:],
                                func=mybir.ActivationFunctionType.Copy)
                        dtp = pp.tile([1, 128], f16, tag="dtp", bufs=DTPBUF)
                        nc.tensor.transpose(out=dtp[:], in_=As[:, F:F + 1],
                                            identity=ident_sb[:])
                        drow = sb.tile([1, 128], f16, tag="drow", bufs=98)
                        nc.scalar.activation(
                            out=drow[:], in_=dtp[:],
                            func=mybir.ActivationFunctionType.Copy)
                        dmax = sb.tile([128, 1], f32, tag="dmax", bufs=4)
                        nc.vector.tensor_scalar_max(out=dmax[:], in0=A[:, F:F + 1],
                                                    scalar1=1e-30)
                        rec = sb.tile([128, 1], f32, tag="rec", bufs=98)
                        nc.vector.reciprocal(out=rec[:], in_=dmax[:])
                        pending.append((pr, m, ATs, drow, rec))
                if g >= LAG_G and pending:
                    if not b16_box:
                        emit_b16()
                    k = -(-len(pending) // max(1, NG - 2 - g))
                    for ent in pending[:k]:
                        stage2(ent)
                    del pending[:k]
            if pending and not b16_box:
                emit_b16()
            for ent in pending:
                stage2(ent)

    nc.finalize()
    return nc


def _prep(x, w, edge_vals, rows, cols, kw):
    """Host-side shard/layout construction. Returns in_maps or None if kw
    is too small for this edge distribution."""
    nch = NW * kw
    gmax = GHW * kw

    order = np.argsort(rows, kind="stable")
    rs = rows[order].astype(np.int64)
    cs = cols[order].astype(np.int64)
    vs = edge_vals[order]

    core = rs // NPC
    lic = rs % NPC
    hwin = lic // WIN
    loc = lic % WIN

    run = core * NW + hwin               # global half-window id, monotone in rs
    nruns = NCORES * NW
    counts = np.bincount(run, minlength=nruns)
    if counts.max() > kw * 128:
        return None
    starts = np.zeros(nruns, np.int64)
    np.cumsum(counts[:-1], out=starts[1:])
    pos = np.arange(len(run)) - starts[run]

    chunk = hwin * kw + pos // 128       # chunk index within the core
    e_part = pos % 128

    locf = np.full((NCORES, 128, nch), -1.0, np.float16)
    valf = np.full((NCORES, 128, nch), -100.0, np.float16)
    colf = np.zeros((NCORES, 128, nch), np.int64)
    locf[core, e_part, chunk] = loc.astype(np.float16)
    valf[core, e_part, chunk] = vs
    colf[core, e_part, chunk] = cs

    x16 = x.astype(np.float16)
    ident = np.eye(128, dtype=np.float16)
    iota = np.ascontiguousarray(np.broadcast_to(
        np.arange(WIN, dtype=np.float16)[None, :, None],
        (128, WIN, gmax))).reshape(128, WIN * gmax)
    w32 = np.ascontiguousarray(w.astype(np.float32))
    in_maps = []
    for c in range(NCORES):
        xs = np.empty((128, nch, F + 1), np.float16)
        xs[:, :, 0:F] = x16[colf[c]]
        xs[:, :, F] = 1.0
        xsh = np.ascontiguousarray(x16[c * NPC:(c + 1) * NPC, :].T)
        in_maps.append({
            "xT": xsh,
            "w_in": w32,
            "ident_in": ident,
            "iota_in": iota,
            "loc_in": np.ascontiguousarray(locf[c]),
            "val_in": np.ascontiguousarray(valf[c]),
            "xs_in": xs.reshape(128, nch * (F + 1)),
        })
    return in_maps


def kernel(x, kernel, edge_vals, rows, cols, nodes_num):
    global _last_kw
    assert int(nodes_num) == N and x.shape == (N, F) and kernel.shape == (F, D)
    kw = 9
    in_maps = _prep(x, kernel, edge_vals, rows, cols, kw)
    while in_maps is None:  # pathological edge distribution: rebuild larger
        kw += 1
        in_maps = _prep(x, kernel, edge_vals, rows, cols, kw)
    _last_kw = kw
    if kw not in _cache:
        _cache[kw] = _build(kw)
    nc = _cache[kw]
    res = run_bass_kernel_spmd(nc, in_maps, core_ids=list(range(NCORES)))
    out = np.concatenate([res.results[c]["out"] for c in range(NCORES)], axis=0)
    return out.astype(np.float32)
